# revision 12
# baseline (speedup 1.0000x reference)
"""NVFP4 block-quantized linear layer (x @ w.T + bias) on 8 Trainium2 cores.

Reference semantics (reference.py): both activations and weights are
block-quantized along K (blocks of 16) to fp4-e2m1 with e4m3 scales
(scale = absmax/6, round-to-nearest), dequantized, then matmul with fp32
accumulation, cast to bf16, plus bf16 bias.

Device strategy (per core, 2-way M x 4-way N grid), v2:
  - PE does ONLY matmuls (4608 x [128x128]@[128x512] bf16, 216ns each):
    stationary = xdqT k-major tile, moving = wdqT k-major slice, so the
    LDWEIGHTS (1 per 3 MMs) hides under the MM stream.
  - all transposes are done by the DMA xbar (dma_start_transpose,
    ~270GB/s measured): quantized bf16 tiles are staged row-major to
    DRAM and read back k-major.
  - W is quantized once, kept SBUF-resident one n-half (1536 cols) at a
    time; x is quantized once, staged to xdq_dram, and its transposed
    k-major super-block tiles are re-read per half.
  - quant chain on DVE; PSUM evac on ACT; bias add on GPSIMD; DMA issue
    split between SYNC (stores + transposed loads) and ACT (plain loads)
    to avoid head-of-line blocking.
"""

import os
import numpy as np
import ml_dtypes

f32 = np.float32
bf16 = ml_dtypes.bfloat16

# ---------------------------------------------------------------------------
# problem geometry (hardcoded; harness calls kernel() with these full shapes)
B, T, K = 2, 4096, 3072
N = 12288
M = B * T                      # 8192
GRID_M, GRID_N = 2, 4          # 8 cores
M_CORE = M // GRID_M           # 4096
N_CORE = N // GRID_N           # 3072
NUM_CORES = GRID_M * GRID_N

KC = K // 128                  # 24 k-chunks
NB = 512                       # matmul moving free dim
HALF = N_CORE // 2             # 1536 resident W columns
NBH = HALF // NB               # 3 n-blocks per half
SB = 512                       # m super-block rows
NSB = M_CORE // SB             # 8 super-blocks
MT_SB = SB // 128              # 4 m-tiles per super-block
WT_HALF = HALF // 128          # 12 w row-tiles per half
QH = K // 2                    # quant half-tile width (1536)
KBH = QH // 16                 # 96 scale blocks per quant half

CH1 = float(1.5 * 2**22)
RCP6 = float(f32(1.0) / f32(6.0))
CM = float(1.5 * 2**20)

_BUILT = None


# ---------------------------------------------------------------------------
def _register_custom_ops():
    """Register the two fp4-rounding custom DVE ops (idempotent)."""
    import concourse.dve_ops as dve_ops
    from concourse.dve_ops import DveOp, OPS, _SUB_OPCODE_FOR_NAME, _CUSTOM_DVE_ROW_BASE
    from concourse.dve_spec import (
        Spec, Src0, Src1, C0, C1, Zero, One, AluOp, Bin,
        maxx, minn, select, lower, _has_src1,
    )
    from concourse.dve_uop import DveOpSpec

    def _norm2(in0, in1):
        in0 = np.asarray(in0)
        in1 = np.asarray(in1)
        if in1.size != in0.size:
            in1 = np.broadcast_to(in1, in0.shape)
        return in0, np.ascontiguousarray(in1).reshape(in0.shape)

    def _ref_fp4_pre(in0, in1, s0, s1, imm2=None):
        in0, in1 = _norm2(in0, in1)
        m = (in0.astype(f32) * in1.astype(f32)).astype(f32)
        s2 = (m * m).astype(f32)
        ch = np.where(
            s2 < f32(4.0), f32(CH1),
            ((f32(1.0) + (s2 >= f32(16.0)).astype(f32)) * f32(1.5 * 2**23)).astype(f32),
        ).astype(f32)
        return (m + ch).astype(f32)

    def _ref_fp4_fin(in0, in1, s0, s1, imm2=None):
        in0, in1 = _norm2(in0, in1)
        qpre = np.ascontiguousarray(in0.astype(f32))
        pe = (qpre.view(np.uint32) & np.uint32(0x7F800000)).view(f32)
        d1 = (qpre - pe).astype(f32)
        q2 = ((d1 + d1).astype(f32) - pe).astype(f32)
        qc = np.maximum(np.minimum(q2, f32(12.0)), f32(-12.0))
        return (qc * in1.astype(f32)).astype(f32)

    def build_pre():
        SIXTEEN = C0 * C0
        Ch2x = C1 + C1
        m = Src0 * Src1
        s2 = m * m
        c2 = s2 >= SIXTEEN
        inner = (c2 + One) * Ch2x
        c1 = s2 < C0
        outer = select(c1, C1, inner)
        return Spec(body=m + outer, reference=_ref_fp4_pre)

    def build_fin():
        pe = Bin(AluOp.BITWISE_AND, Src0, C0)
        d1 = Src0 - pe
        q2 = (d1 + d1) - pe
        qc = maxx(minn(q2, C1), Zero - C1)
        return Spec(body=qc * Src1, reference=_ref_fp4_fin)

    def register(name, spec):
        if name in _SUB_OPCODE_FOR_NAME:
            for op in OPS:
                if op.name == name:
                    return op
            raise RuntimeError(name)
        row = _CUSTOM_DVE_ROW_BASE + len(OPS)
        assert row < 0x20
        shas = {}
        for ver in ("v3", "v4"):
            try:
                uops = lower(spec, ver=ver)
            except Exception:
                continue
            shas[ver] = DveOpSpec(
                name=name, opcode=row, uops=uops, rd1_en=_has_src1(spec)
            ).sha(ver)
        op = DveOp(name, spec, subdim=False, uops_sha=shas)
        OPS.append(op)
        _SUB_OPCODE_FOR_NAME[name] = row
        dve_ops.CUSTOM_DVE_SPECS[name] = spec
        return op

    return register("FP4_PRE_ANT", build_pre()), register("FP4_FIN_ANT", build_fin())


# ---------------------------------------------------------------------------
def _patch_tile_drain():
    """The TileContext tail drain attaches one sem-wait per live logical
    processor to a single SP Drain instruction; this walrus build caps sync
    waits per instruction at 2 ("Too many sync wait commands").  Split the
    overflow waits onto preceding single-wait SP nops (sound: all waits still
    complete before the post-drain all-engine barrier / sem reset)."""
    from concourse import tile as tile_mod
    import concourse.mybir as mybir
    from concourse.vector_clock import ScopedClock

    if getattr(tile_mod.TileContext, "_ant_drain_patched", False):
        return

    def _drain_and_barrier(self, tick_clock, wait_clock):
        nc = self.nc
        probe = nc.sync.nop()
        wait_clock.add_sem_waits(
            probe.ins, ScopedClock({None: tick_clock.global_clock})
        )
        si = probe.ins.sync_info
        waits = list(si.on_wait) if si is not None and si.on_wait else []
        if len(waits) > 1:
            probe.ins.sync_info = mybir.SyncInfo(
                on_wait=waits[:1],
                on_update=list(si.on_update) if si.on_update else [],
            )
            for w in waits[1:]:
                extra = nc.sync.nop()
                extra.ins.sync_info = mybir.SyncInfo(on_wait=[w], on_update=[])
        nc.sync.drain()

        nc.all_engine_barrier()
        assert self.sems is not None
        popped = nc._tile_sem_poison_stack.pop()
        assert popped is self._sem_poison
        nc.clear_and_free_semaphores(list(self.sems.allocated().values()))
        nc.all_engine_barrier()

    tile_mod.TileContext._drain_and_barrier = _drain_and_barrier
    tile_mod.TileContext._ant_drain_patched = True


def _split_excess_waits(nc, max_waits=1):
    """This walrus build rejects instructions carrying more than `max_waits`
    sem waits ("Too many sync wait commands").  Move overflow waits onto
    same-engine NoOp instructions inserted immediately before the offender —
    per-engine program order makes this semantically identical."""
    import concourse.mybir as mybir

    ctr = [0]
    for f in nc.m.functions:
        for blk in f.blocks:
            il = blk.instructions
            out = []
            changed = False
            for ins in il:
                si = ins.sync_info
                waits = list(si.on_wait) if si is not None and si.on_wait else []
                if len(waits) > max_waits:
                    changed = True
                    extra = waits[:-max_waits]
                    for i0 in range(0, len(extra), max_waits):
                        nop = mybir.InstNoOp(
                            name=f"I-waitsplit-{ctr[0]}", ins=[], outs=[])
                        ctr[0] += 1
                        nop.engine = ins.engine
                        nop.sync_info = mybir.SyncInfo(
                            on_wait=extra[i0:i0 + max_waits], on_update=[])
                        out.append(nop)
                    ins.sync_info = mybir.SyncInfo(
                        on_wait=waits[-max_waits:],
                        on_update=list(si.on_update) if si.on_update else [],
                    )
                out.append(ins)
            if changed:
                blk.instructions = out


# ---------------------------------------------------------------------------
def build_nc(debug=False, postprocess=True):
    """Build the per-core Bass program (SPMD: same program on every core)."""
    import concourse.bass as bass
    import concourse.mybir as mybir
    from concourse import tile
    from contextlib import ExitStack

    fp4_pre, fp4_fin = _register_custom_ops()
    _patch_tile_drain()

    nc = bass.Bass("TRN2", target_bir_lowering=False, debug=debug,
                   num_devices=NUM_CORES)
    dt = mybir.dt
    Alu = mybir.AluOpType

    x_d = nc.dram_tensor("x", [M_CORE, K], dt.float32, kind="ExternalInput")
    w_d = nc.dram_tensor("w", [N_CORE, K], dt.float32, kind="ExternalInput")
    b_d = nc.dram_tensor("bias", [N_CORE], dt.bfloat16, kind="ExternalInput")
    out_d = nc.dram_tensor("out", [M_CORE, N_CORE], dt.bfloat16,
                           kind="ExternalOutput")

    with tile.TileContext(nc) as tc, ExitStack() as ctx:
        dram = ctx.enter_context(tc.tile_pool(name="dram", bufs=1, space="DRAM"))
        # quant input halves [128, 1536] f32; 8 bufs = two super-blocks in
        # flight (pipelining) at 6KiB each
        xin = ctx.enter_context(tc.tile_pool(name="xin", bufs=7))
        xdqp = ctx.enter_context(tc.tile_pool(name="xdqp", bufs=2))
        blk = ctx.enter_context(tc.tile_pool(name="blk", bufs=1))
        wres = ctx.enter_context(tc.tile_pool(name="wres", bufs=1))
        xres = ctx.enter_context(tc.tile_pool(name="xres", bufs=2))
        outp = ctx.enter_context(tc.tile_pool(name="outp", bufs=12))
        cst = ctx.enter_context(tc.tile_pool(name="cst", bufs=1))
        psmm = ctx.enter_context(tc.tile_pool(name="psmm", bufs=1, space="PSUM"))

        xdq_dram = dram.tile([M_CORE, K], dt.bfloat16)
        wdq_dram = dram.tile([N_CORE, K], dt.bfloat16)

        # constants
        inf_t = cst.tile([128, 1], dt.float32, tag="inf")
        nc.vector.memset(inf_t[:, :], float("inf"))
        bias_t = cst.tile([128, N_CORE], dt.bfloat16, tag="bias")
        nc.sync.dma_start(
            out=bias_t[:, :],
            in_=b_d[:].unsqueeze(0).broadcast_to([128, N_CORE]),
        )

        KB = K // 16  # 192 scale blocks per full tile

        def quant_tile(r0, src_d, dst_dram, tag):
            """Quantize rows [r0, r0+128) of src_d (f32 [rows, K]) into
            dst_dram (bf16).  Data moves in two half-tiles of 1536 columns
            (SBUF economy); the scale chain runs once per tile on [128,192].
            Loads on ACT queue, stores on SYNC, chain on DVE with the two
            mults offloaded to GPSIMD."""
            xdq = xdqp.tile([128, K], dt.bfloat16, tag="xdq", name=f"xdq_{tag}")
            xts = []
            bm = blk.tile([128, KB], dt.float32, tag="bm", name="bm")
            for h in range(2):
                xt = xin.tile([128, QH], dt.float32, tag="xin",
                              name=f"xin_{tag}_{h}")
                nc.scalar.dma_start(
                    out=xt[:, :], in_=src_d[r0:r0 + 128, h * QH:(h + 1) * QH])
                xts.append(xt)
                nc.vector.tensor_reduce(
                    bm[:, h * KBH:(h + 1) * KBH],
                    xt[:, :].rearrange("p (b e) -> p b e", e=16),
                    axis=mybir.AxisListType.X, op=Alu.max,
                    apply_absolute_value=True,
                )
            sraw = blk.tile([128, KB], dt.float32, tag="sraw", name="sraw")
            nc.vector.tensor_scalar(
                sraw[:, :], bm[:, :], RCP6, float(2.0**-9), Alu.mult, Alu.max)
            peb = blk.tile([128, KB], dt.float32, tag="peb", name="peb")
            nc.vector.tensor_scalar(
                peb[:, :].bitcast(dt.int32), sraw[:, :].bitcast(dt.int32),
                0x7F800000, None, Alu.bitwise_and)
            pe2 = blk.tile([128, KB], dt.float32, tag="pe2", name="pe2")
            nc.vector.tensor_scalar_max(pe2[:, :], peb[:, :], float(2.0**-6))
            # exact 1/pe2 for powers of two: bits(1/p) = 0x7F000000 - bits(p)
            pinv = blk.tile([128, KB], dt.float32, tag="pinv", name="pinv")
            nc.vector.tensor_scalar(
                pinv[:, :].bitcast(dt.int32), pe2[:, :].bitcast(dt.int32),
                -1, 0x7F000000, Alu.mult, Alu.add)
            u = blk.tile([128, KB], dt.float32, tag="u", name="u")
            nc.gpsimd.tensor_tensor(u[:, :], sraw[:, :], pinv[:, :], Alu.mult)
            wq = blk.tile([128, KB], dt.float32, tag="wq", name="wq")
            nc.vector.tensor_scalar(wq[:, :], u[:, :], CM, -CM, Alu.add, Alu.add)
            s = blk.tile([128, KB], dt.float32, tag="s", name="s")
            nc.gpsimd.tensor_tensor(s[:, :], wq[:, :], pe2[:, :], Alu.mult)
            sh = blk.tile([128, KB], dt.float32, tag="sh", name="sh")
            nc.vector.tensor_scalar_mul(sh[:, :], s[:, :], 0.5)
            rinv = blk.tile([128, KB], dt.float32, tag="rinv", name="rinv")
            nc.vector.reciprocal(rinv[:, :], s[:, :])

            for h in range(2):
                x3 = xts[h][:, :].rearrange("p (b e) -> p b e", e=16)
                rv = rinv[:, h * KBH:(h + 1) * KBH]
                sv = sh[:, h * KBH:(h + 1) * KBH]
                # fp4 round: PRE in-place over xt, FIN into the xdq slice
                nc.vector._custom_dve(
                    fp4_pre, out=x3, in0=x3,
                    in1=rv.unsqueeze(2).broadcast_to([128, KBH, 16]),
                    s0=4.0, s1=CH1,
                )
                xdq3 = xdq[:, h * QH:(h + 1) * QH].rearrange(
                    "p (b e) -> p b e", e=16)
                nc.vector._custom_dve(
                    fp4_fin, out=xdq3, in0=x3,
                    in1=sv.unsqueeze(2).broadcast_to([128, KBH, 16]),
                    s0=inf_t[:, 0:1], s1=12.0,
                )
            nc.sync.dma_start(out=dst_dram[r0:r0 + 128, :], in_=xdq[:, :])

        wtiles = {}

        def wq_nb(half, nb):
            """Quantize one W n-block (4 row-tiles) and fill its resident
            wdqT tile via transposed read."""
            for t in range(4):
                r0 = half * HALF + nb * NB + t * 128
                quant_tile(r0, w_d, wdq_dram, f"w{r0 // 128}")
            wt = wres.tile([128, KC, NB], dt.bfloat16, tag=f"wres{nb}",
                           name=f"wres{half}_{nb}")
            r0 = half * HALF + nb * NB
            nc.sync.dma_start_transpose(wt[:, :, :], wdq_dram[r0:r0 + NB, :])
            wtiles[nb] = wt

        def xq(sb):
            for t in range(MT_SB):
                quant_tile(sb * SB + t * 128, x_d, xdq_dram, f"x{sb}_{t}")

        xtiles = {}

        def xresT(sb):
            xT = xres.tile([128, KC, SB], dt.bfloat16, tag="xres",
                           name=f"xres{sb}")
            nc.sync.dma_start_transpose(
                xT[:, :, :], xdq_dram[sb * SB:(sb + 1) * SB, :])
            xtiles[sb] = xT

        _chunk_ctr = [0]
        from collections import deque
        _deferred = deque()

        def _flush_deferred(keep=2):
            # Emit bias+store for chunks >= `keep` behind: delaying these
            # keeps gpsimd bias ops (which wait on evacs) from head-of-line
            # blocking the quant-chain mults also queued on gpsimd.
            while len(_deferred) > keep:
                for och, c0, r0 in _deferred.popleft():
                    nc.gpsimd.tensor_tensor(
                        och[:, :], och[:, :], bias_t[:, c0:c0 + NB], Alu.add)
                    nc.sync.dma_start(
                        out=out_d[r0:r0 + 128, c0:c0 + NB], in_=och[:, :])

        def chunk(half, sb, nb):
            """One (super-block, n-block) MM chunk: 4 m-tiles x 24 kc
            accumulations, then evac per m-tile (bias+store deferred)."""
            par = _chunk_ctr[0] % 2
            _chunk_ctr[0] += 1
            xT = xtiles[sb]
            wt = wtiles[nb]
            pm = [psmm.tile([128, NB], dt.float32, tag=f"mm{par}_{mt}",
                            name=f"pmm{half}_{sb}_{nb}_{mt}")
                  for mt in range(MT_SB)]
            for kc in range(KC):
                for mt in range(MT_SB):
                    nc.tensor.matmul(
                        pm[mt][:, :],
                        xT[:, kc, mt * 128:(mt + 1) * 128],
                        wt[:, kc, :],
                        start=(kc == 0), stop=(kc == KC - 1),
                    )
            group = []
            for mt in range(MT_SB):
                och = outp.tile([128, NB], dt.bfloat16, tag="ostage",
                                name=f"ost{half}_{sb}_{nb}_{mt}")
                nc.scalar.copy(och[:, :], pm[mt][:, :])
                c0 = half * HALF + nb * NB
                r0 = sb * SB + mt * 128
                group.append((och, c0, r0))
            _deferred.append(group)
            _flush_deferred(keep=2)

        # ------------------- emission schedule -------------------
        # Interleaved so that (a) DVE streams w-nb0, x0, w-nb1, x1, w-nb2,
        # x2..x7, wB without gaps, (b) PE chunks are emitted in the order
        # their inputs become available, (c) xres slot rotation (bufs=2)
        # only ever waits on already-emitted chunks.
        wq_nb(0, 0)
        xq(0); xresT(0)
        wq_nb(0, 1)
        xq(1); xresT(1)
        chunk(0, 0, 0)
        wq_nb(0, 2)
        chunk(0, 0, 1)
        xq(2)
        chunk(0, 1, 0)
        chunk(0, 1, 1)
        chunk(0, 0, 2)
        xresT(2)
        chunk(0, 1, 2)
        xq(3); xresT(3)
        chunk(0, 2, 0)
        chunk(0, 2, 1)
        xq(4)
        chunk(0, 2, 2)
        xresT(4)
        chunk(0, 3, 0)
        chunk(0, 3, 1)
        xq(5)
        chunk(0, 3, 2)
        xresT(5)
        chunk(0, 4, 0)
        chunk(0, 4, 1)
        xq(6)
        chunk(0, 4, 2)
        xresT(6)
        chunk(0, 5, 0)
        chunk(0, 5, 1)
        xq(7)
        chunk(0, 5, 2)
        xresT(7)
        chunk(0, 6, 0)
        chunk(0, 6, 1)
        chunk(0, 6, 2)
        chunk(0, 7, 0)
        wq_nb(1, 0)     # wresT-B nb0 fires as soon as chunk(0,7,0) is done
        chunk(0, 7, 1)
        wq_nb(1, 1)
        chunk(0, 7, 2)
        wq_nb(1, 2)
        xresT(0)        # phase-B prefetch (slot of sb6)
        xresT(1)
        for sb in range(NSB):
            for nb in range(NBH):
                chunk(1, sb, nb)
            if sb + 2 < NSB:
                xresT(sb + 2)
        _flush_deferred(keep=0)

    if postprocess:
        _split_excess_waits(nc)
        # Raw Bass skips the ISA-byte encoding pass (Bacc.compile runs it);
        # without it custom-DVE/extended insts ship empty .instr -> walrus
        # "ISA wrong length".
        mybir.codegen_inst_isa_subclasses(nc)
    return nc


# ---------------------------------------------------------------------------
def _get_built():
    global _BUILT
    if _BUILT is None:
        _BUILT = build_nc()
    return _BUILT


def make_in_maps(x2, w, b):
    in_maps = []
    for c in range(NUM_CORES):
        mi, nj = divmod(c, GRID_N)
        in_maps.append({
            "x": x2[mi * M_CORE:(mi + 1) * M_CORE],
            "w": w[nj * N_CORE:(nj + 1) * N_CORE],
            "bias": b[nj * N_CORE:(nj + 1) * N_CORE],
        })
    return in_maps


def kernel(x, weight, bias):
    """Full-input entry point: x [2,4096,3072] f32, weight [12288,3072] f32,
    bias [12288] bf16 -> out [2,4096,12288] bf16."""
    from concourse.bass_utils import run_bass_kernel_spmd

    nc = _get_built()
    x2 = np.ascontiguousarray(np.asarray(x, dtype=f32).reshape(M, K))
    w = np.ascontiguousarray(np.asarray(weight, dtype=f32))
    b = np.asarray(bias)
    if b.dtype != bf16:
        if b.dtype.itemsize == 2 and b.dtype.kind in "Vu":
            b = b.view(bf16)
        else:
            b = b.astype(bf16)

    res = run_bass_kernel_spmd(nc, make_in_maps(x2, w, b),
                               list(range(NUM_CORES)))
    out = np.empty((M, N), dtype=bf16)
    for c in range(NUM_CORES):
        mi, nj = divmod(c, GRID_N)
        out[mi * M_CORE:(mi + 1) * M_CORE, nj * N_CORE:(nj + 1) * N_CORE] = (
            np.asarray(res.results[c]["out"]).astype(bf16, copy=False)
        )
    return out.reshape(B, T, N)


# revision 14
# speedup vs baseline: 1.0614x; 1.0614x over previous
"""NVFP4 block-quantized linear layer (x @ w.T + bias) on 8 Trainium2 cores.

Reference semantics (reference.py): both activations and weights are
block-quantized along K (blocks of 16) to fp4-e2m1 with e4m3 scales
(scale = absmax/6, round-to-nearest), dequantized, then matmul with fp32
accumulation, cast to bf16, plus bf16 bias.

Device strategy (per core, 2-way M x 4-way N grid), v2:
  - PE does ONLY matmuls (4608 x [128x128]@[128x512] bf16, 216ns each):
    stationary = xdqT k-major tile, moving = wdqT k-major slice, so the
    LDWEIGHTS (1 per 3 MMs) hides under the MM stream.
  - all transposes are done by the DMA xbar (dma_start_transpose,
    ~270GB/s measured): quantized bf16 tiles are staged row-major to
    DRAM and read back k-major.
  - W is quantized once, kept SBUF-resident one n-half (1536 cols) at a
    time; x is quantized once, staged to xdq_dram, and its transposed
    k-major super-block tiles are re-read per half.
  - quant chain on DVE; PSUM evac on ACT; bias add on GPSIMD; DMA issue
    split between SYNC (stores + transposed loads) and ACT (plain loads)
    to avoid head-of-line blocking.
"""

import os
import numpy as np
import ml_dtypes

f32 = np.float32
bf16 = ml_dtypes.bfloat16

# ---------------------------------------------------------------------------
# problem geometry (hardcoded; harness calls kernel() with these full shapes)
B, T, K = 2, 4096, 3072
N = 12288
M = B * T                      # 8192
GRID_M, GRID_N = 2, 4          # 8 cores
M_CORE = M // GRID_M           # 4096
N_CORE = N // GRID_N           # 3072
NUM_CORES = GRID_M * GRID_N

KC = K // 128                  # 24 k-chunks
NB = 512                       # matmul moving free dim
HALF = N_CORE // 2             # 1536 resident W columns
NBH = HALF // NB               # 3 n-blocks per half
SB = 512                       # m super-block rows
NSB = M_CORE // SB             # 8 super-blocks
MT_SB = SB // 128              # 4 m-tiles per super-block
WT_HALF = HALF // 128          # 12 w row-tiles per half
QH = K // 2                    # quant half-tile width (1536)
KBH = QH // 16                 # 96 scale blocks per quant half

CH1 = float(1.5 * 2**22)
RCP6 = float(f32(1.0) / f32(6.0))
CM = float(1.5 * 2**20)

_BUILT = None


# ---------------------------------------------------------------------------
def _register_custom_ops():
    """Register the two fp4-rounding custom DVE ops (idempotent)."""
    import concourse.dve_ops as dve_ops
    from concourse.dve_ops import DveOp, OPS, _SUB_OPCODE_FOR_NAME, _CUSTOM_DVE_ROW_BASE
    from concourse.dve_spec import (
        Spec, Src0, Src1, C0, C1, Zero, One, AluOp, Bin,
        maxx, minn, select, lower, _has_src1,
    )
    from concourse.dve_uop import DveOpSpec

    def _norm2(in0, in1):
        in0 = np.asarray(in0)
        in1 = np.asarray(in1)
        if in1.size != in0.size:
            in1 = np.broadcast_to(in1, in0.shape)
        return in0, np.ascontiguousarray(in1).reshape(in0.shape)

    def _ref_fp4_pre(in0, in1, s0, s1, imm2=None):
        in0, in1 = _norm2(in0, in1)
        m = (in0.astype(f32) * in1.astype(f32)).astype(f32)
        s2 = (m * m).astype(f32)
        ch = np.where(
            s2 < f32(4.0), f32(CH1),
            ((f32(1.0) + (s2 >= f32(16.0)).astype(f32)) * f32(1.5 * 2**23)).astype(f32),
        ).astype(f32)
        return (m + ch).astype(f32)

    def _ref_fp4_fin(in0, in1, s0, s1, imm2=None):
        in0, in1 = _norm2(in0, in1)
        qpre = np.ascontiguousarray(in0.astype(f32))
        pe = (qpre.view(np.uint32) & np.uint32(0x7F800000)).view(f32)
        d1 = (qpre - pe).astype(f32)
        q2 = ((d1 + d1).astype(f32) - pe).astype(f32)
        qc = np.maximum(np.minimum(q2, f32(12.0)), f32(-12.0))
        return (qc * in1.astype(f32)).astype(f32)

    def build_pre():
        SIXTEEN = C0 * C0
        Ch2x = C1 + C1
        m = Src0 * Src1
        s2 = m * m
        c2 = s2 >= SIXTEEN
        inner = (c2 + One) * Ch2x
        c1 = s2 < C0
        outer = select(c1, C1, inner)
        return Spec(body=m + outer, reference=_ref_fp4_pre)

    def build_fin():
        pe = Bin(AluOp.BITWISE_AND, Src0, C0)
        d1 = Src0 - pe
        q2 = (d1 + d1) - pe
        qc = maxx(minn(q2, C1), Zero - C1)
        return Spec(body=qc * Src1, reference=_ref_fp4_fin)

    def register(name, spec):
        if name in _SUB_OPCODE_FOR_NAME:
            for op in OPS:
                if op.name == name:
                    return op
            raise RuntimeError(name)
        row = _CUSTOM_DVE_ROW_BASE + len(OPS)
        assert row < 0x20
        shas = {}
        for ver in ("v3", "v4"):
            try:
                uops = lower(spec, ver=ver)
            except Exception:
                continue
            shas[ver] = DveOpSpec(
                name=name, opcode=row, uops=uops, rd1_en=_has_src1(spec)
            ).sha(ver)
        op = DveOp(name, spec, subdim=False, uops_sha=shas)
        OPS.append(op)
        _SUB_OPCODE_FOR_NAME[name] = row
        dve_ops.CUSTOM_DVE_SPECS[name] = spec
        return op

    return register("FP4_PRE_ANT", build_pre()), register("FP4_FIN_ANT", build_fin())


# ---------------------------------------------------------------------------
def _patch_tile_drain():
    """The TileContext tail drain attaches one sem-wait per live logical
    processor to a single SP Drain instruction; this walrus build caps sync
    waits per instruction at 2 ("Too many sync wait commands").  Split the
    overflow waits onto preceding single-wait SP nops (sound: all waits still
    complete before the post-drain all-engine barrier / sem reset)."""
    from concourse import tile as tile_mod
    import concourse.mybir as mybir
    from concourse.vector_clock import ScopedClock

    if getattr(tile_mod.TileContext, "_ant_drain_patched", False):
        return

    def _drain_and_barrier(self, tick_clock, wait_clock):
        nc = self.nc
        probe = nc.sync.nop()
        wait_clock.add_sem_waits(
            probe.ins, ScopedClock({None: tick_clock.global_clock})
        )
        si = probe.ins.sync_info
        waits = list(si.on_wait) if si is not None and si.on_wait else []
        if len(waits) > 1:
            probe.ins.sync_info = mybir.SyncInfo(
                on_wait=waits[:1],
                on_update=list(si.on_update) if si.on_update else [],
            )
            for w in waits[1:]:
                extra = nc.sync.nop()
                extra.ins.sync_info = mybir.SyncInfo(on_wait=[w], on_update=[])
        nc.sync.drain()

        nc.all_engine_barrier()
        assert self.sems is not None
        popped = nc._tile_sem_poison_stack.pop()
        assert popped is self._sem_poison
        nc.clear_and_free_semaphores(list(self.sems.allocated().values()))
        nc.all_engine_barrier()

    tile_mod.TileContext._drain_and_barrier = _drain_and_barrier
    tile_mod.TileContext._ant_drain_patched = True


def _split_excess_waits(nc, max_waits=1):
    """This walrus build rejects instructions carrying more than `max_waits`
    sem waits ("Too many sync wait commands").  Move overflow waits onto
    same-engine NoOp instructions inserted immediately before the offender —
    per-engine program order makes this semantically identical."""
    import concourse.mybir as mybir

    ctr = [0]
    for f in nc.m.functions:
        for blk in f.blocks:
            il = blk.instructions
            out = []
            changed = False
            for ins in il:
                si = ins.sync_info
                waits = list(si.on_wait) if si is not None and si.on_wait else []
                if len(waits) > max_waits:
                    changed = True
                    extra = waits[:-max_waits]
                    for i0 in range(0, len(extra), max_waits):
                        nop = mybir.InstNoOp(
                            name=f"I-waitsplit-{ctr[0]}", ins=[], outs=[])
                        ctr[0] += 1
                        nop.engine = ins.engine
                        nop.sync_info = mybir.SyncInfo(
                            on_wait=extra[i0:i0 + max_waits], on_update=[])
                        out.append(nop)
                    ins.sync_info = mybir.SyncInfo(
                        on_wait=waits[-max_waits:],
                        on_update=list(si.on_update) if si.on_update else [],
                    )
                out.append(ins)
            if changed:
                blk.instructions = out


# ---------------------------------------------------------------------------
def build_nc(debug=False, postprocess=True):
    """Build the per-core Bass program (SPMD: same program on every core)."""
    import concourse.bass as bass
    import concourse.mybir as mybir
    from concourse import tile
    from contextlib import ExitStack

    fp4_pre, fp4_fin = _register_custom_ops()
    _patch_tile_drain()

    nc = bass.Bass("TRN2", target_bir_lowering=False, debug=debug,
                   num_devices=NUM_CORES)
    dt = mybir.dt
    Alu = mybir.AluOpType

    x_d = nc.dram_tensor("x", [M_CORE, K], dt.float32, kind="ExternalInput")
    w_d = nc.dram_tensor("w", [N_CORE, K], dt.float32, kind="ExternalInput")
    b_d = nc.dram_tensor("bias", [N_CORE], dt.bfloat16, kind="ExternalInput")
    out_d = nc.dram_tensor("out", [M_CORE, N_CORE], dt.bfloat16,
                           kind="ExternalOutput")

    with tile.TileContext(nc) as tc, ExitStack() as ctx:
        dram = ctx.enter_context(tc.tile_pool(name="dram", bufs=1, space="DRAM"))
        # quant input halves [128, 1536] f32; 8 bufs = two super-blocks in
        # flight (pipelining) at 6KiB each
        xin = ctx.enter_context(tc.tile_pool(name="xin", bufs=7))
        xdqp = ctx.enter_context(tc.tile_pool(name="xdqp", bufs=2))
        blk = ctx.enter_context(tc.tile_pool(name="blk", bufs=1))
        wres = ctx.enter_context(tc.tile_pool(name="wres", bufs=1))
        xres = ctx.enter_context(tc.tile_pool(name="xres", bufs=2))
        outp = ctx.enter_context(tc.tile_pool(name="outp", bufs=12))
        cst = ctx.enter_context(tc.tile_pool(name="cst", bufs=1))
        psmm = ctx.enter_context(tc.tile_pool(name="psmm", bufs=1, space="PSUM"))

        xdq_dram = dram.tile([M_CORE, K], dt.bfloat16)
        wdq_dram = dram.tile([N_CORE, K], dt.bfloat16)

        # constants
        inf_t = cst.tile([128, 1], dt.float32, tag="inf")
        nc.vector.memset(inf_t[:, :], float("inf"))
        bias_t = cst.tile([128, N_CORE], dt.bfloat16, tag="bias")
        nc.sync.dma_start(
            out=bias_t[:, :],
            in_=b_d[:].unsqueeze(0).broadcast_to([128, N_CORE]),
        )

        KB = K // 16  # 192 scale blocks per full tile

        def quant_tile(r0, src_d, dst_dram, tag):
            """Quantize rows [r0, r0+128) of src_d (f32 [rows, K]) into
            dst_dram (bf16).  Data moves in two half-tiles of 1536 columns
            (SBUF economy); the scale chain runs once per tile on [128,192].
            Loads on ACT queue, stores on SYNC, chain on DVE with the two
            mults offloaded to GPSIMD."""
            xdq = xdqp.tile([128, K], dt.bfloat16, tag="xdq", name=f"xdq_{tag}")
            xts = []
            bm = blk.tile([128, KB], dt.float32, tag="bm", name="bm")
            for h in range(2):
                xt = xin.tile([128, QH], dt.float32, tag="xin",
                              name=f"xin_{tag}_{h}")
                nc.scalar.dma_start(
                    out=xt[:, :], in_=src_d[r0:r0 + 128, h * QH:(h + 1) * QH])
                xts.append(xt)
                nc.vector.tensor_reduce(
                    bm[:, h * KBH:(h + 1) * KBH],
                    xt[:, :].rearrange("p (b e) -> p b e", e=16),
                    axis=mybir.AxisListType.X, op=Alu.max,
                    apply_absolute_value=True,
                )
            sraw = blk.tile([128, KB], dt.float32, tag="sraw", name="sraw")
            nc.vector.tensor_scalar(
                sraw[:, :], bm[:, :], RCP6, float(2.0**-9), Alu.mult, Alu.max)
            peb = blk.tile([128, KB], dt.float32, tag="peb", name="peb")
            nc.vector.tensor_scalar(
                peb[:, :].bitcast(dt.int32), sraw[:, :].bitcast(dt.int32),
                0x7F800000, None, Alu.bitwise_and)
            pe2 = blk.tile([128, KB], dt.float32, tag="pe2", name="pe2")
            nc.vector.tensor_scalar_max(pe2[:, :], peb[:, :], float(2.0**-6))
            # exact 1/pe2 for powers of two: bits(1/p) = 0x7F000000 - bits(p)
            pinv = blk.tile([128, KB], dt.float32, tag="pinv", name="pinv")
            nc.vector.tensor_scalar(
                pinv[:, :].bitcast(dt.int32), pe2[:, :].bitcast(dt.int32),
                -1, 0x7F000000, Alu.mult, Alu.add)
            u = blk.tile([128, KB], dt.float32, tag="u", name="u")
            nc.gpsimd.tensor_tensor(u[:, :], sraw[:, :], pinv[:, :], Alu.mult)
            wq = blk.tile([128, KB], dt.float32, tag="wq", name="wq")
            nc.vector.tensor_scalar(wq[:, :], u[:, :], CM, -CM, Alu.add, Alu.add)
            s = blk.tile([128, KB], dt.float32, tag="s", name="s")
            nc.gpsimd.tensor_tensor(s[:, :], wq[:, :], pe2[:, :], Alu.mult)
            sh = blk.tile([128, KB], dt.float32, tag="sh", name="sh")
            nc.vector.tensor_scalar_mul(sh[:, :], s[:, :], 0.5)
            rinv = blk.tile([128, KB], dt.float32, tag="rinv", name="rinv")
            nc.vector.reciprocal(rinv[:, :], s[:, :])

            for h in range(2):
                x3 = xts[h][:, :].rearrange("p (b e) -> p b e", e=16)
                rv = rinv[:, h * KBH:(h + 1) * KBH]
                sv = sh[:, h * KBH:(h + 1) * KBH]
                # fp4 round: PRE in-place over xt, FIN into the xdq slice
                nc.vector._custom_dve(
                    fp4_pre, out=x3, in0=x3,
                    in1=rv.unsqueeze(2).broadcast_to([128, KBH, 16]),
                    s0=4.0, s1=CH1,
                )
                xdq3 = xdq[:, h * QH:(h + 1) * QH].rearrange(
                    "p (b e) -> p b e", e=16)
                nc.vector._custom_dve(
                    fp4_fin, out=xdq3, in0=x3,
                    in1=sv.unsqueeze(2).broadcast_to([128, KBH, 16]),
                    s0=inf_t[:, 0:1], s1=12.0,
                )
            nc.sync.dma_start(out=dst_dram[r0:r0 + 128, :], in_=xdq[:, :])

        wtiles = {}

        def wq_nb(half, nb):
            """Quantize one W n-block (4 row-tiles) and fill its resident
            wdqT tile via transposed read."""
            for t in range(4):
                r0 = half * HALF + nb * NB + t * 128
                quant_tile(r0, w_d, wdq_dram, f"w{r0 // 128}")
            wt = wres.tile([128, KC, NB], dt.bfloat16, tag=f"wres{nb}",
                           name=f"wres{half}_{nb}")
            r0 = half * HALF + nb * NB
            nc.sync.dma_start_transpose(wt[:, :, :], wdq_dram[r0:r0 + NB, :])
            wtiles[nb] = wt

        def xq(sb):
            for t in range(MT_SB):
                quant_tile(sb * SB + t * 128, x_d, xdq_dram, f"x{sb}_{t}")

        xtiles = {}

        def xresT(sb):
            xT = xres.tile([128, KC, SB], dt.bfloat16, tag="xres",
                           name=f"xres{sb}")
            nc.sync.dma_start_transpose(
                xT[:, :, :], xdq_dram[sb * SB:(sb + 1) * SB, :])
            xtiles[sb] = xT

        _chunk_ctr = [0]
        from collections import deque
        _evac_q = deque()     # chunks whose PSUM evac is not yet emitted
        _deferred = deque()   # evac'd chunks whose bias+store is pending

        def _flush_evacs(keep=1):
            # Evacs wait on the chunk's matmuls; emitting them one chunk
            # late means they execute with ~zero wait and never head-of-line
            # block the x/w input loads also queued on ACT.
            while len(_evac_q) > keep:
                pm, c0, sb = _evac_q.popleft()
                group = []
                for mt in range(MT_SB):
                    och = outp.tile([128, NB], dt.bfloat16, tag="ostage",
                                    name=f"ost{c0}_{sb}_{mt}")
                    nc.scalar.copy(och[:, :], pm[mt][:, :])
                    group.append((och, c0, sb * SB + mt * 128))
                _deferred.append(group)

        def _flush_deferred(keep=1):
            # Same trick for the gpsimd bias adds (vs the chain mults).
            while len(_deferred) > keep:
                for och, c0, r0 in _deferred.popleft():
                    nc.gpsimd.tensor_tensor(
                        och[:, :], och[:, :], bias_t[:, c0:c0 + NB], Alu.add)
                    nc.sync.dma_start(
                        out=out_d[r0:r0 + 128, c0:c0 + NB], in_=och[:, :])

        def chunk(half, sb, nb):
            """One (super-block, n-block) MM chunk: 4 m-tiles x 24 kc
            accumulations.  Evac / bias / store of earlier chunks are
            emitted here, pipeline-deferred."""
            par = _chunk_ctr[0] % 2
            _chunk_ctr[0] += 1
            xT = xtiles[sb]
            wt = wtiles[nb]
            pm = [psmm.tile([128, NB], dt.float32, tag=f"mm{par}_{mt}",
                            name=f"pmm{half}_{sb}_{nb}_{mt}")
                  for mt in range(MT_SB)]
            for kc in range(KC):
                for mt in range(MT_SB):
                    nc.tensor.matmul(
                        pm[mt][:, :],
                        xT[:, kc, mt * 128:(mt + 1) * 128],
                        wt[:, kc, :],
                        start=(kc == 0), stop=(kc == KC - 1),
                    )
            _evac_q.append((pm, half * HALF + nb * NB, sb))
            _flush_evacs(keep=1)
            _flush_deferred(keep=1)

        def _flush_all():
            _flush_evacs(keep=0)
            _flush_deferred(keep=0)

        # ------------------- emission schedule -------------------
        # Interleaved so that (a) DVE streams w-nb0, x0, w-nb1, x1, w-nb2,
        # x2..x7, wB without gaps, (b) PE chunks are emitted in the order
        # their inputs become available, (c) xres slot rotation (bufs=2)
        # only ever waits on already-emitted chunks.
        wq_nb(0, 0)
        xq(0); xresT(0)
        wq_nb(0, 1)
        xq(1); xresT(1)
        chunk(0, 0, 0)
        wq_nb(0, 2)
        chunk(0, 0, 1)
        xq(2)
        chunk(0, 1, 0)
        chunk(0, 1, 1)
        chunk(0, 0, 2)
        xresT(2)
        chunk(0, 1, 2)
        xq(3); xresT(3)
        chunk(0, 2, 0)
        chunk(0, 2, 1)
        xq(4)
        chunk(0, 2, 2)
        xresT(4)
        chunk(0, 3, 0)
        chunk(0, 3, 1)
        xq(5)
        chunk(0, 3, 2)
        xresT(5)
        chunk(0, 4, 0)
        chunk(0, 4, 1)
        xq(6)
        chunk(0, 4, 2)
        xresT(6)
        chunk(0, 5, 0)
        chunk(0, 5, 1)
        xq(7)
        chunk(0, 5, 2)
        xresT(7)
        chunk(0, 6, 0)
        chunk(0, 6, 1)
        chunk(0, 6, 2)
        chunk(0, 7, 0)
        wq_nb(1, 0)     # wresT-B nb0 fires as soon as chunk(0,7,0) is done
        chunk(0, 7, 1)
        wq_nb(1, 1)
        chunk(0, 7, 2)
        wq_nb(1, 2)
        xresT(0)        # phase-B prefetch (slot of sb6)
        xresT(1)
        for sb in range(NSB):
            for nb in range(NBH):
                chunk(1, sb, nb)
            if sb + 2 < NSB:
                xresT(sb + 2)
        _flush_all()

    if postprocess:
        _split_excess_waits(nc)
        # Raw Bass skips the ISA-byte encoding pass (Bacc.compile runs it);
        # without it custom-DVE/extended insts ship empty .instr -> walrus
        # "ISA wrong length".
        mybir.codegen_inst_isa_subclasses(nc)
    return nc


# ---------------------------------------------------------------------------
def _get_built():
    global _BUILT
    if _BUILT is None:
        _BUILT = build_nc()
    return _BUILT


def make_in_maps(x2, w, b):
    in_maps = []
    for c in range(NUM_CORES):
        mi, nj = divmod(c, GRID_N)
        in_maps.append({
            "x": x2[mi * M_CORE:(mi + 1) * M_CORE],
            "w": w[nj * N_CORE:(nj + 1) * N_CORE],
            "bias": b[nj * N_CORE:(nj + 1) * N_CORE],
        })
    return in_maps


def kernel(x, weight, bias):
    """Full-input entry point: x [2,4096,3072] f32, weight [12288,3072] f32,
    bias [12288] bf16 -> out [2,4096,12288] bf16."""
    from concourse.bass_utils import run_bass_kernel_spmd

    nc = _get_built()
    x2 = np.ascontiguousarray(np.asarray(x, dtype=f32).reshape(M, K))
    w = np.ascontiguousarray(np.asarray(weight, dtype=f32))
    b = np.asarray(bias)
    if b.dtype != bf16:
        if b.dtype.itemsize == 2 and b.dtype.kind in "Vu":
            b = b.view(bf16)
        else:
            b = b.astype(bf16)

    res = run_bass_kernel_spmd(nc, make_in_maps(x2, w, b),
                               list(range(NUM_CORES)))
    out = np.empty((M, N), dtype=bf16)
    for c in range(NUM_CORES):
        mi, nj = divmod(c, GRID_N)
        out[mi * M_CORE:(mi + 1) * M_CORE, nj * N_CORE:(nj + 1) * N_CORE] = (
            np.asarray(res.results[c]["out"]).astype(bf16, copy=False)
        )
    return out.reshape(B, T, N)


# revision 16
# speedup vs baseline: 1.0654x; 1.0037x over previous
"""NVFP4 block-quantized linear layer (x @ w.T + bias) on 8 Trainium2 cores.

Reference semantics (reference.py): both activations and weights are
block-quantized along K (blocks of 16) to fp4-e2m1 with e4m3 scales
(scale = absmax/6, round-to-nearest), dequantized, then matmul with fp32
accumulation, cast to bf16, plus bf16 bias.

Device strategy (per core, 2-way M x 4-way N grid), v2:
  - PE does ONLY matmuls (4608 x [128x128]@[128x512] bf16, 216ns each):
    stationary = xdqT k-major tile, moving = wdqT k-major slice, so the
    LDWEIGHTS (1 per 3 MMs) hides under the MM stream.
  - all transposes are done by the DMA xbar (dma_start_transpose,
    ~270GB/s measured): quantized bf16 tiles are staged row-major to
    DRAM and read back k-major.
  - W is quantized once, kept SBUF-resident one n-half (1536 cols) at a
    time; x is quantized once, staged to xdq_dram, and its transposed
    k-major super-block tiles are re-read per half.
  - quant chain on DVE; PSUM evac on ACT; bias add on GPSIMD; DMA issue
    split between SYNC (stores + transposed loads) and ACT (plain loads)
    to avoid head-of-line blocking.
"""

import os
import numpy as np
import ml_dtypes

f32 = np.float32
bf16 = ml_dtypes.bfloat16

# ---------------------------------------------------------------------------
# problem geometry (hardcoded; harness calls kernel() with these full shapes)
B, T, K = 2, 4096, 3072
N = 12288
M = B * T                      # 8192
GRID_M, GRID_N = 2, 4          # 8 cores
M_CORE = M // GRID_M           # 4096
N_CORE = N // GRID_N           # 3072
NUM_CORES = GRID_M * GRID_N

KC = K // 128                  # 24 k-chunks
NB = 512                       # matmul moving free dim
HALF = N_CORE // 2             # 1536 resident W columns
NBH = HALF // NB               # 3 n-blocks per half
SB = 512                       # m super-block rows
NSB = M_CORE // SB             # 8 super-blocks
MT_SB = SB // 128              # 4 m-tiles per super-block
WT_HALF = HALF // 128          # 12 w row-tiles per half
QH = K // 2                    # quant half-tile width (1536)
KBH = QH // 16                 # 96 scale blocks per quant half

CH1 = float(1.5 * 2**22)
RCP6 = float(f32(1.0) / f32(6.0))
CM = float(1.5 * 2**20)

_BUILT = None


# ---------------------------------------------------------------------------
def _register_custom_ops():
    """Register the two fp4-rounding custom DVE ops (idempotent)."""
    import concourse.dve_ops as dve_ops
    from concourse.dve_ops import DveOp, OPS, _SUB_OPCODE_FOR_NAME, _CUSTOM_DVE_ROW_BASE
    from concourse.dve_spec import (
        Spec, Src0, Src1, C0, C1, Zero, One, AluOp, Bin,
        maxx, minn, select, lower, _has_src1,
    )
    from concourse.dve_uop import DveOpSpec

    def _norm2(in0, in1):
        in0 = np.asarray(in0)
        in1 = np.asarray(in1)
        if in1.size != in0.size:
            in1 = np.broadcast_to(in1, in0.shape)
        return in0, np.ascontiguousarray(in1).reshape(in0.shape)

    def _ref_fp4_pre(in0, in1, s0, s1, imm2=None):
        in0, in1 = _norm2(in0, in1)
        m = (in0.astype(f32) * in1.astype(f32)).astype(f32)
        s2 = (m * m).astype(f32)
        ch = np.where(
            s2 < f32(4.0), f32(CH1),
            ((f32(1.0) + (s2 >= f32(16.0)).astype(f32)) * f32(1.5 * 2**23)).astype(f32),
        ).astype(f32)
        return (m + ch).astype(f32)

    def _ref_fp4_fin(in0, in1, s0, s1, imm2=None):
        in0, in1 = _norm2(in0, in1)
        qpre = np.ascontiguousarray(in0.astype(f32))
        pe = (qpre.view(np.uint32) & np.uint32(0x7F800000)).view(f32)
        d1 = (qpre - pe).astype(f32)
        q2 = ((d1 + d1).astype(f32) - pe).astype(f32)
        qc = np.maximum(np.minimum(q2, f32(12.0)), f32(-12.0))
        return (qc * in1.astype(f32)).astype(f32)

    def build_pre():
        SIXTEEN = C0 * C0
        Ch2x = C1 + C1
        m = Src0 * Src1
        s2 = m * m
        c2 = s2 >= SIXTEEN
        inner = (c2 + One) * Ch2x
        c1 = s2 < C0
        outer = select(c1, C1, inner)
        return Spec(body=m + outer, reference=_ref_fp4_pre)

    def build_fin():
        pe = Bin(AluOp.BITWISE_AND, Src0, C0)
        d1 = Src0 - pe
        q2 = (d1 + d1) - pe
        qc = maxx(minn(q2, C1), Zero - C1)
        return Spec(body=qc * Src1, reference=_ref_fp4_fin)

    def register(name, spec):
        if name in _SUB_OPCODE_FOR_NAME:
            for op in OPS:
                if op.name == name:
                    return op
            raise RuntimeError(name)
        row = _CUSTOM_DVE_ROW_BASE + len(OPS)
        assert row < 0x20
        shas = {}
        for ver in ("v3", "v4"):
            try:
                uops = lower(spec, ver=ver)
            except Exception:
                continue
            shas[ver] = DveOpSpec(
                name=name, opcode=row, uops=uops, rd1_en=_has_src1(spec)
            ).sha(ver)
        op = DveOp(name, spec, subdim=False, uops_sha=shas)
        OPS.append(op)
        _SUB_OPCODE_FOR_NAME[name] = row
        dve_ops.CUSTOM_DVE_SPECS[name] = spec
        return op

    return register("FP4_PRE_ANT", build_pre()), register("FP4_FIN_ANT", build_fin())


# ---------------------------------------------------------------------------
def _patch_tile_drain():
    """The TileContext tail drain attaches one sem-wait per live logical
    processor to a single SP Drain instruction; this walrus build caps sync
    waits per instruction at 2 ("Too many sync wait commands").  Split the
    overflow waits onto preceding single-wait SP nops (sound: all waits still
    complete before the post-drain all-engine barrier / sem reset)."""
    from concourse import tile as tile_mod
    import concourse.mybir as mybir
    from concourse.vector_clock import ScopedClock

    if getattr(tile_mod.TileContext, "_ant_drain_patched", False):
        return

    def _drain_and_barrier(self, tick_clock, wait_clock):
        nc = self.nc
        probe = nc.sync.nop()
        wait_clock.add_sem_waits(
            probe.ins, ScopedClock({None: tick_clock.global_clock})
        )
        si = probe.ins.sync_info
        waits = list(si.on_wait) if si is not None and si.on_wait else []
        if len(waits) > 1:
            probe.ins.sync_info = mybir.SyncInfo(
                on_wait=waits[:1],
                on_update=list(si.on_update) if si.on_update else [],
            )
            for w in waits[1:]:
                extra = nc.sync.nop()
                extra.ins.sync_info = mybir.SyncInfo(on_wait=[w], on_update=[])
        nc.sync.drain()

        nc.all_engine_barrier()
        assert self.sems is not None
        popped = nc._tile_sem_poison_stack.pop()
        assert popped is self._sem_poison
        nc.clear_and_free_semaphores(list(self.sems.allocated().values()))
        nc.all_engine_barrier()

    tile_mod.TileContext._drain_and_barrier = _drain_and_barrier
    tile_mod.TileContext._ant_drain_patched = True


def _split_excess_waits(nc, max_waits=1):
    """This walrus build rejects instructions carrying more than `max_waits`
    sem waits ("Too many sync wait commands").  Move overflow waits onto
    same-engine NoOp instructions inserted immediately before the offender —
    per-engine program order makes this semantically identical."""
    import concourse.mybir as mybir

    ctr = [0]
    for f in nc.m.functions:
        for blk in f.blocks:
            il = blk.instructions
            out = []
            changed = False
            for ins in il:
                si = ins.sync_info
                waits = list(si.on_wait) if si is not None and si.on_wait else []
                if len(waits) > max_waits:
                    changed = True
                    extra = waits[:-max_waits]
                    for i0 in range(0, len(extra), max_waits):
                        nop = mybir.InstNoOp(
                            name=f"I-waitsplit-{ctr[0]}", ins=[], outs=[])
                        ctr[0] += 1
                        nop.engine = ins.engine
                        nop.sync_info = mybir.SyncInfo(
                            on_wait=extra[i0:i0 + max_waits], on_update=[])
                        out.append(nop)
                    ins.sync_info = mybir.SyncInfo(
                        on_wait=waits[-max_waits:],
                        on_update=list(si.on_update) if si.on_update else [],
                    )
                out.append(ins)
            if changed:
                blk.instructions = out


# ---------------------------------------------------------------------------
def build_nc(debug=False, postprocess=True):
    """Build the per-core Bass program (SPMD: same program on every core)."""
    import concourse.bass as bass
    import concourse.mybir as mybir
    from concourse import tile
    from contextlib import ExitStack

    fp4_pre, fp4_fin = _register_custom_ops()
    _patch_tile_drain()

    nc = bass.Bass("TRN2", target_bir_lowering=False, debug=debug,
                   num_devices=NUM_CORES)
    dt = mybir.dt
    Alu = mybir.AluOpType

    x_d = nc.dram_tensor("x", [M_CORE, K], dt.float32, kind="ExternalInput")
    w_d = nc.dram_tensor("w", [N_CORE, K], dt.float32, kind="ExternalInput")
    b_d = nc.dram_tensor("bias", [N_CORE], dt.bfloat16, kind="ExternalInput")
    out_d = nc.dram_tensor("out", [M_CORE, N_CORE], dt.bfloat16,
                           kind="ExternalOutput")

    with tile.TileContext(nc) as tc, ExitStack() as ctx:
        dram = ctx.enter_context(tc.tile_pool(name="dram", bufs=1, space="DRAM"))
        # quant input halves [128, 1536] f32; 8 bufs = two super-blocks in
        # flight (pipelining) at 6KiB each
        xin = ctx.enter_context(tc.tile_pool(name="xin", bufs=7))
        xdqp = ctx.enter_context(tc.tile_pool(name="xdqp", bufs=2))
        blk = ctx.enter_context(tc.tile_pool(name="blk", bufs=1))
        wres = ctx.enter_context(tc.tile_pool(name="wres", bufs=1))
        xres = ctx.enter_context(tc.tile_pool(name="xres", bufs=2))
        outp = ctx.enter_context(tc.tile_pool(name="outp", bufs=12))
        cst = ctx.enter_context(tc.tile_pool(name="cst", bufs=1))
        psmm = ctx.enter_context(tc.tile_pool(name="psmm", bufs=1, space="PSUM"))

        xdq_dram = dram.tile([M_CORE, K], dt.bfloat16)
        wdq_dram = dram.tile([N_CORE, K], dt.bfloat16)

        # constants
        inf_t = cst.tile([128, 1], dt.float32, tag="inf")
        nc.vector.memset(inf_t[:, :], float("inf"))
        bias_t = cst.tile([128, N_CORE], dt.bfloat16, tag="bias")
        nc.sync.dma_start(
            out=bias_t[:, :],
            in_=b_d[:].unsqueeze(0).broadcast_to([128, N_CORE]),
        )

        KB = K // 16  # 192 scale blocks per full tile

        def quant_tile(r0, src_d, dst_dram, tag):
            """Quantize rows [r0, r0+128) of src_d (f32 [rows, K]) into
            dst_dram (bf16).  Data moves in two half-tiles of 1536 columns
            (SBUF economy); the scale chain runs once per tile on [128,192].
            Loads on ACT queue, stores on SYNC, chain on DVE with the two
            mults offloaded to GPSIMD."""
            xdq = xdqp.tile([128, K], dt.bfloat16, tag="xdq", name=f"xdq_{tag}")
            xts = []
            bm = blk.tile([128, KB], dt.float32, tag="bm", name="bm")
            for h in range(2):
                xt = xin.tile([128, QH], dt.float32, tag="xin",
                              name=f"xin_{tag}_{h}")
                nc.scalar.dma_start(
                    out=xt[:, :], in_=src_d[r0:r0 + 128, h * QH:(h + 1) * QH])
                xts.append(xt)
                nc.vector.tensor_reduce(
                    bm[:, h * KBH:(h + 1) * KBH],
                    xt[:, :].rearrange("p (b e) -> p b e", e=16),
                    axis=mybir.AxisListType.X, op=Alu.max,
                    apply_absolute_value=True,
                )
            sraw = blk.tile([128, KB], dt.float32, tag="sraw", name="sraw")
            nc.vector.tensor_scalar(
                sraw[:, :], bm[:, :], RCP6, float(2.0**-9), Alu.mult, Alu.max)
            peb = blk.tile([128, KB], dt.float32, tag="peb", name="peb")
            nc.vector.tensor_scalar(
                peb[:, :].bitcast(dt.int32), sraw[:, :].bitcast(dt.int32),
                0x7F800000, None, Alu.bitwise_and)
            pe2 = blk.tile([128, KB], dt.float32, tag="pe2", name="pe2")
            nc.vector.tensor_scalar_max(pe2[:, :], peb[:, :], float(2.0**-6))
            # exact 1/pe2 for powers of two: bits(1/p) = 0x7F000000 - bits(p)
            pinv = blk.tile([128, KB], dt.float32, tag="pinv", name="pinv")
            nc.vector.tensor_scalar(
                pinv[:, :].bitcast(dt.int32), pe2[:, :].bitcast(dt.int32),
                -1, 0x7F000000, Alu.mult, Alu.add)
            u = blk.tile([128, KB], dt.float32, tag="u", name="u")
            nc.gpsimd.tensor_tensor(u[:, :], sraw[:, :], pinv[:, :], Alu.mult)
            wq = blk.tile([128, KB], dt.float32, tag="wq", name="wq")
            nc.vector.tensor_scalar(wq[:, :], u[:, :], CM, -CM, Alu.add, Alu.add)
            s = blk.tile([128, KB], dt.float32, tag="s", name="s")
            nc.gpsimd.tensor_tensor(s[:, :], wq[:, :], pe2[:, :], Alu.mult)
            sh = blk.tile([128, KB], dt.float32, tag="sh", name="sh")
            nc.vector.tensor_scalar_mul(sh[:, :], s[:, :], 0.5)
            rinv = blk.tile([128, KB], dt.float32, tag="rinv", name="rinv")
            nc.vector.reciprocal(rinv[:, :], s[:, :])

            for h in range(2):
                x3 = xts[h][:, :].rearrange("p (b e) -> p b e", e=16)
                rv = rinv[:, h * KBH:(h + 1) * KBH]
                sv = sh[:, h * KBH:(h + 1) * KBH]
                # fp4 round: PRE in-place over xt, FIN into the xdq slice
                nc.vector._custom_dve(
                    fp4_pre, out=x3, in0=x3,
                    in1=rv.unsqueeze(2).broadcast_to([128, KBH, 16]),
                    s0=4.0, s1=CH1,
                )
                xdq3 = xdq[:, h * QH:(h + 1) * QH].rearrange(
                    "p (b e) -> p b e", e=16)
                nc.vector._custom_dve(
                    fp4_fin, out=xdq3, in0=x3,
                    in1=sv.unsqueeze(2).broadcast_to([128, KBH, 16]),
                    s0=inf_t[:, 0:1], s1=12.0,
                )
            nc.sync.dma_start(out=dst_dram[r0:r0 + 128, :], in_=xdq[:, :])

        wtiles = {}

        def wq_nb(half, nb):
            """Quantize one W n-block (4 row-tiles) and fill its resident
            wdqT tile via transposed read."""
            for t in range(4):
                r0 = half * HALF + nb * NB + t * 128
                quant_tile(r0, w_d, wdq_dram, f"w{r0 // 128}")
            wt = wres.tile([128, KC, NB], dt.bfloat16, tag=f"wres{nb}",
                           name=f"wres{half}_{nb}")
            r0 = half * HALF + nb * NB
            nc.sync.dma_start_transpose(wt[:, :, :], wdq_dram[r0:r0 + NB, :])
            wtiles[nb] = wt

        def xq(sb):
            for t in range(MT_SB):
                quant_tile(sb * SB + t * 128, x_d, xdq_dram, f"x{sb}_{t}")

        xtiles = {}

        def xresT(sb):
            xT = xres.tile([128, KC, SB], dt.bfloat16, tag="xres",
                           name=f"xres{sb}")
            nc.sync.dma_start_transpose(
                xT[:, :, :], xdq_dram[sb * SB:(sb + 1) * SB, :])
            xtiles[sb] = xT

        _chunk_ctr = [0]
        from collections import deque
        _evac_q = deque()     # chunks whose PSUM evac is not yet emitted
        _deferred = deque()   # evac'd chunks whose bias+store is pending

        def _flush_evacs(keep=1):
            # Evacs wait on the chunk's matmuls; emitting them one chunk
            # late means they execute with ~zero wait and never head-of-line
            # block the x/w input loads also queued on ACT.
            while len(_evac_q) > keep:
                pm, c0, sb = _evac_q.popleft()
                group = []
                for mt in range(MT_SB):
                    och = outp.tile([128, NB], dt.bfloat16, tag="ostage",
                                    name=f"ost{c0}_{sb}_{mt}")
                    nc.scalar.copy(och[:, :], pm[mt][:, :])
                    group.append((och, c0, sb * SB + mt * 128))
                _deferred.append(group)

        def _flush_deferred(keep=1):
            # Same trick for the gpsimd bias adds (vs the chain mults).
            while len(_deferred) > keep:
                for och, c0, r0 in _deferred.popleft():
                    nc.gpsimd.tensor_tensor(
                        och[:, :], och[:, :], bias_t[:, c0:c0 + NB], Alu.add)
                    nc.sync.dma_start(
                        out=out_d[r0:r0 + 128, c0:c0 + NB], in_=och[:, :])

        def chunk(half, sb, nb):
            """One (super-block, n-block) MM chunk: 4 m-tiles x 24 kc
            accumulations.  Evac / bias / store of earlier chunks are
            emitted here, pipeline-deferred."""
            par = _chunk_ctr[0] % 2
            _chunk_ctr[0] += 1
            xT = xtiles[sb]
            wt = wtiles[nb]
            pm = [psmm.tile([128, NB], dt.float32, tag=f"mm{par}_{mt}",
                            name=f"pmm{half}_{sb}_{nb}_{mt}")
                  for mt in range(MT_SB)]
            for kc in range(KC):
                for mt in range(MT_SB):
                    nc.tensor.matmul(
                        pm[mt][:, :],
                        xT[:, kc, mt * 128:(mt + 1) * 128],
                        wt[:, kc, :],
                        start=(kc == 0), stop=(kc == KC - 1),
                    )
            _evac_q.append((pm, half * HALF + nb * NB, sb))
            _flush_evacs(keep=1)
            _flush_deferred(keep=1)

        def _flush_all():
            _flush_evacs(keep=0)
            _flush_deferred(keep=0)

        # ------------------- emission schedule -------------------
        # Interleaved so that (a) DVE streams w-nb0, x0, w-nb1, x1, w-nb2,
        # x2..x7, wB without gaps, (b) PE chunks are emitted in the order
        # their inputs become available, (c) xres slot rotation (bufs=2)
        # only ever waits on already-emitted chunks.
        wq_nb(0, 0)
        xq(0); xresT(0)
        wq_nb(0, 1)
        xq(1); xresT(1)
        chunk(0, 0, 0)
        wq_nb(0, 2)
        chunk(0, 0, 1)
        xq(2)
        chunk(0, 1, 0)
        chunk(0, 1, 1)
        chunk(0, 0, 2)
        xresT(2)
        chunk(0, 1, 2)
        xq(3); xresT(3)
        chunk(0, 2, 0)
        chunk(0, 2, 1)
        xq(4)
        chunk(0, 2, 2)
        xresT(4)
        chunk(0, 3, 0)
        chunk(0, 3, 1)
        xq(5)
        chunk(0, 3, 2)
        xresT(5)
        chunk(0, 4, 0)
        chunk(0, 4, 1)
        xq(6)
        chunk(0, 4, 2)
        xresT(6)
        chunk(0, 5, 0)
        chunk(0, 5, 1)
        xq(7)
        chunk(0, 5, 2)
        xresT(7)
        chunk(0, 6, 0)
        chunk(0, 6, 1)
        chunk(0, 6, 2)
        chunk(0, 7, 0)
        wq_nb(1, 0)     # wresT-B nb0 fires as soon as chunk(0,7,0) is done
        chunk(0, 7, 1)
        wq_nb(1, 1)
        chunk(0, 7, 2)
        wq_nb(1, 2)
        xresT(0)        # phase-B prefetch (slot of sb6)
        xresT(1)
        for sb in range(NSB):
            for nb in range(NBH):
                chunk(1, sb, nb)
            if sb + 2 < NSB:
                xresT(sb + 2)
        _flush_all()

    if postprocess:
        _split_excess_waits(nc)
        # Raw Bass skips the ISA-byte encoding pass (Bacc.compile runs it);
        # without it custom-DVE/extended insts ship empty .instr -> walrus
        # "ISA wrong length".
        mybir.codegen_inst_isa_subclasses(nc)
    return nc


# ---------------------------------------------------------------------------
def _get_built():
    global _BUILT
    if _BUILT is None:
        _BUILT = build_nc()
    return _BUILT


def make_in_maps(x2, w, b):
    in_maps = []
    for c in range(NUM_CORES):
        mi, nj = divmod(c, GRID_N)
        in_maps.append({
            "x": x2[mi * M_CORE:(mi + 1) * M_CORE],
            "w": w[nj * N_CORE:(nj + 1) * N_CORE],
            "bias": b[nj * N_CORE:(nj + 1) * N_CORE],
        })
    return in_maps


def kernel(x, weight, bias):
    """Full-input entry point: x [2,4096,3072] f32, weight [12288,3072] f32,
    bias [12288] bf16 -> out [2,4096,12288] bf16."""
    from concourse.bass_utils import run_bass_kernel_spmd

    nc = _get_built()
    x2 = np.ascontiguousarray(np.asarray(x, dtype=f32).reshape(M, K))
    w = np.ascontiguousarray(np.asarray(weight, dtype=f32))
    b = np.asarray(bias)
    if b.dtype != bf16:
        if b.dtype.itemsize == 2 and b.dtype.kind in "Vu":
            b = b.view(bf16)
        else:
            b = b.astype(bf16)

    res = run_bass_kernel_spmd(nc, make_in_maps(x2, w, b),
                               list(range(NUM_CORES)))
    out = np.empty((M, N), dtype=bf16)
    for c in range(NUM_CORES):
        mi, nj = divmod(c, GRID_N)
        out[mi * M_CORE:(mi + 1) * M_CORE, nj * N_CORE:(nj + 1) * N_CORE] = (
            np.asarray(res.results[c]["out"]).astype(bf16, copy=False)
        )
    return out.reshape(B, T, N)
